# revision 33
# baseline (speedup 1.0000x reference)
"""Trainium2 Bass kernel for GQA attention (B=2, S=2048, HID=2048, H=16, G=4, D=128).

Sharding: 8 cores = 2 batches x 4 kv-groups. Core c handles batch c//4 and
kv-group c%4 (4 q heads + 1 kv head). Wq/Wk/Wv column-sharded by head group,
Wo row-sharded; per-core partial outputs are summed on the host per batch
(the unshard / all-reduce step).

v5: bf16 operands; fully pipelined per-chunk schedule (proj_c -> attn_c ->
outproj_c, k/v pass first); V projected directly in transposed layout
(lhsT = x); full-partition RoPE with sign-folded sin table; all DRAM operands
host-pre-tiled into SBUF layout so every DMA moves 2-4KB contiguous runs per
partition; bf16 output summed on the host in fp32.
"""

import os
import sys

sys.path.insert(0, "/opt/trn_rl_repo")

import numpy as np

B, S, HID = 2, 2048, 2048
H, G, D = 16, 4, 128
REP = H // G  # q heads per kv head = 4
NEG = -1e30
SCALE = 1.0 / np.sqrt(np.float32(D))

NKT = HID // 128  # 16 contraction tiles for projections
NSC = S // 512  # 4 s-chunks of 512
NST = S // 128  # 16 s-tiles of 128


def _emit(nc, tc, aps):
    """Emit the per-core program. aps: dict of DRAM APs."""
    from contextlib import ExitStack

    from concourse import mybir

    f32 = mybir.dt.float32
    bf16 = mybir.dt.bfloat16
    fp8 = mybir.dt.float8e4
    DR = mybir.MatmulPerfMode.DoubleRow
    Exp = mybir.ActivationFunctionType.Exp

    # pre-tiled DRAM operands (partition dim first, contiguous free dims)
    xT = aps["xT"]  # [128, 4c, 4p, 4t, 512] bf16
    wq, wk, wv, wo = aps["wq"], aps["wk"], aps["wv"], aps["wo"]
    cosT, sinT = aps["cosT"], aps["sinT"]  # [128, 2048] bf16
    mask, ones, ones8 = aps["mask"], aps["ones"], aps["ones8"]
    out = aps["out"]

    lo = slice(0, 64)
    hi = slice(64, 128)

    with ExitStack() as ctx:
        persist = ctx.enter_context(tc.tile_pool(name="persist", bufs=1))

        # constants
        mask_t = persist.tile([128, 128], f32, tag="mask", name="mask_t")
        ones_t = persist.tile([128, 128], bf16, tag="ones", name="ones_t")
        ones8_t = persist.tile([128, 2, 128], fp8, tag="ones8", name="ones8_t")

        # weights
        wq_p = [
            persist.tile([128, 4, 512], bf16, tag=f"wq{i}", name=f"wq{i}")
            for i in range(4)
        ]
        wk_sb = persist.tile([128, 16, 128], bf16, tag="wk", name="wk_sb")
        wv_sb = persist.tile([128, 16, 128], bf16, tag="wv", name="wv_sb")
        wo_p = [
            persist.tile([128, 1, 2048], bf16, tag=f"wo{i}", name=f"wo{i}")
            for i in range(4)
        ]
        cos_p = [
            persist.tile([128, 1024], bf16, tag=f"cos{i}", name=f"cos{i}")
            for i in range(2)
        ]
        sin_p = [
            persist.tile([128, 1024], bf16, tag=f"sin{i}", name=f"sin{i}")
            for i in range(2)
        ]

        # persistent activations
        qt = [
            persist.tile([128, S], bf16, tag=f"qt{h}", name=f"qt{h}")
            for h in range(REP)
        ]
        kt = persist.tile([128, S], bf16, tag="kt", name="kt")
        vs = persist.tile([128, NST, 128], bf16, tag="vs", name="vs")
        aot = [
            persist.tile([128, S], bf16, tag=f"aot{h}", name=f"aot{h}")
            for h in range(REP)
        ]

        xpool = ctx.enter_context(tc.tile_pool(name="xsl", bufs=8))
        quad = ctx.enter_context(tc.tile_pool(name="quad", bufs=4, space="PSUM"))
        pb = ctx.enter_context(tc.tile_pool(name="pb", bufs=4, space="PSUM"))
        epool = ctx.enter_context(tc.tile_pool(name="es", bufs=5))
        e8pool = ctx.enter_context(tc.tile_pool(name="es8", bufs=4))
        rpool = ctx.enter_context(tc.tile_pool(name="rope", bufs=4))
        rcp = ctx.enter_context(tc.tile_pool(name="rc", bufs=2))
        ocp = ctx.enter_context(tc.tile_pool(name="oc", bufs=4))

        # ---------------- startup DMA sequence ----------------
        # x slabs: per chunk, 4 piece-tiles of [128, 4, 512]
        xs = [
            [
                xpool.tile([128, 4, 512], bf16, tag="x", name=f"xs{c}_{p}")
                for p in range(4)
            ]
            for c in range(NSC)
        ]

        def dma_x_chunk(c, eng):
            for p in range(4):
                eng.dma_start(xs[c][p][:], xT[:, c, p, :, :])

        def dma_x_piece(c, p, eng):
            eng.dma_start(xs[c][p][:], xT[:, c, p, :, :])

        # scalar queue: wk/wv (gate the first pass), trig, mask
        nc.scalar.dma_start(wk_sb[:], wk)
        nc.scalar.dma_start(wv_sb[:], wv)
        nc.scalar.dma_start(cos_p[0][:], cosT[:, 0:1024])
        nc.scalar.dma_start(sin_p[0][:], sinT[:, 0:1024])
        nc.scalar.dma_start(cos_p[1][:], cosT[:, 1024:2048])
        nc.scalar.dma_start(sin_p[1][:], sinT[:, 1024:2048])
        nc.scalar.dma_start(mask_t[:], mask)
        # sync queue: front of x chunk0, first wq halves
        dma_x_piece(0, 0, nc.sync)
        dma_x_piece(0, 1, nc.sync)
        nc.sync.dma_start(wq_p[0][:], wq[:, 0:4, :])
        nc.sync.dma_start(wq_p[1][:], wq[:, 4:8, :])
        # gpsimd queue: back of x chunk0, rest of wq, ones, x chunk1, wo
        dma_x_piece(0, 2, nc.gpsimd)
        dma_x_piece(0, 3, nc.gpsimd)
        nc.gpsimd.dma_start(wq_p[2][:], wq[:, 8:12, :])
        nc.gpsimd.dma_start(wq_p[3][:], wq[:, 12:16, :])
        nc.gpsimd.dma_start(ones_t[:], ones)
        nc.gpsimd.dma_start(ones8_t[:], ones8)
        dma_x_chunk(1, nc.gpsimd)
        for m in range(4):
            nc.gpsimd.dma_start(wo_p[m][:], wo[:, m : m + 1, :])

        def rope(acc, dest, cs_off, piece):
            """dest = acc*cos + swap_half(acc)*sin_signed  (sin pre-negated in
            its low half on the host)."""
            cp = cos_p[piece]
            sp_ = sin_p[piece]
            co = slice(cs_off, cs_off + 512)
            tmp_a = rpool.tile([128, 512], f32, tag="ra", name="tmp_a")
            tmp_b = rpool.tile([128, 512], f32, tag="rb", name="tmp_b")
            nc.vector.tensor_mul(tmp_b[lo, :], acc[hi, :], sp_[lo, co])
            nc.vector.tensor_mul(tmp_b[hi, :], acc[lo, :], sp_[hi, co])
            nc.vector.tensor_mul(tmp_a[:], acc[:], cp[:, co])
            nc.vector.tensor_add(dest, tmp_a[:], tmp_b[:])

        for c in range(NSC):
            cs = slice(c * 512, (c + 1) * 512)
            piece = c // 2
            cs_off = (c % 2) * 512

            # ---------------- projections ----------------
            # pass A: k and v^T (so RoPE-k and V land first for attn)
            k_acc = quad.tile([128, 512], f32, tag="qd", name="k_acc")
            for t in range(NKT):
                nc.tensor.matmul(
                    k_acc[:],
                    lhsT=wk_sb[:, t, :],
                    rhs=xs[c][t // 4][:, t % 4, :],
                    start=(t == 0),
                    stop=(t == NKT - 1),
                )
            vT_acc = quad.tile([128, 4, 128], f32, tag="qd", name="vT_acc")
            for i in range(4):
                for t in range(NKT):
                    nc.tensor.matmul(
                        vT_acc[:, i, :],
                        lhsT=xs[c][t // 4][:, t % 4, i * 128 : (i + 1) * 128],
                        rhs=wv_sb[:, t, :],
                        start=(t == 0),
                        stop=(t == NKT - 1),
                    )
            rope(k_acc, kt[:, cs], cs_off, piece)
            nc.scalar.copy(vs[:, 4 * c : 4 * c + 4, :], vT_acc[:])

            # pass B: q0, q1 ; pass C: q2, q3
            for pair in range(2):
                q_acc = [
                    quad.tile([128, 512], f32, tag="qd", name=f"q_acc{pair}{j}")
                    for j in range(2)
                ]
                for t in range(NKT):
                    for j in range(2):
                        h = 2 * pair + j
                        nc.tensor.matmul(
                            q_acc[j][:],
                            lhsT=wq_p[t // 4][:, t % 4, h * 128 : (h + 1) * 128],
                            rhs=xs[c][t // 4][:, t % 4, :],
                            start=(t == 0),
                            stop=(t == NKT - 1),
                        )
                for j in range(2):
                    h = 2 * pair + j
                    rope(q_acc[j], qt[h][:, cs], cs_off, piece)

            # prefetch x for chunk c+2
            if c + 2 < NSC:
                dma_x_chunk(c + 2, nc.sync)

            # ---------------- attention for q-chunk c ----------------
            for h in range(REP):
                av = pb.tile([128, 512], f32, tag="pb", name="av")
                den = pb.tile([128, 512], f32, tag="pb", name="den")
                es_slabs = []
                es8_slabs = []
                for kb in range(c + 1):
                    diag = kb == c
                    es = epool.tile([128, 4, 512], bf16, tag="es", name="es")
                    es_slabs.append(es)
                    for j in range(4):
                        i = kb * 4 + j
                        j0 = j * 128 if diag else 0
                        sp_t = pb.tile([128, 512], f32, tag="pb", name="sp_t")
                        nc.tensor.matmul(
                            sp_t[:, j0:512],
                            lhsT=kt[:, i * 128 : (i + 1) * 128],
                            rhs=qt[h][:, c * 512 + j0 : (c + 1) * 512],
                            start=True,
                            stop=True,
                        )
                        if diag:
                            nc.vector.tensor_add(
                                sp_t[:, j * 128 : (j + 1) * 128],
                                sp_t[:, j * 128 : (j + 1) * 128],
                                mask_t[:],
                            )
                        nc.scalar.activation(
                            es[:, j, j0:512],
                            sp_t[:, j0:512],
                            Exp,
                            scale=float(SCALE),
                        )
                    if not diag:
                        # fp8 shadow copy for the DoubleRow denominator; cast
                        # engine alternates so the trace ranks their speed
                        es8 = e8pool.tile([128, 4, 512], fp8, tag="es8", name="es8")
                        es8_slabs.append(es8)
                        nc.vector.tensor_copy(es8[:], es[:])
                    for j in range(4):
                        i = kb * 4 + j
                        j0 = j * 128 if diag else 0
                        nc.tensor.matmul(
                            av[:, j0:512],
                            lhsT=vs[:, i, :],
                            rhs=es[:, j, j0:512],
                            start=(i == 0),
                            stop=(i == 4 * c + 3),
                        )
                # denominator phase: DoubleRow fp8 for full groups, bf16 for
                # the diagonal; kept off the sc/av critical path
                for kb in range(c):
                    for pair in range(2):
                        nc.tensor.matmul(
                            den[:, :],
                            lhsT=ones8_t[:],
                            rhs=es8_slabs[kb][:, 2 * pair : 2 * pair + 2, :],
                            start=(kb == 0 and pair == 0),
                            stop=False,
                            perf_mode=DR,
                        )
                for j in range(4):
                    j0 = j * 128
                    nc.tensor.matmul(
                        den[:, j0:512],
                        lhsT=ones_t[:],
                        rhs=es_slabs[c][:, j, j0:512],
                        start=(c == 0 and j == 0),
                        stop=(j == 3),
                    )
                rc = rcp.tile([128, 512], f32, tag="rc", name="rc")
                nc.vector.reciprocal_approx_fast(rc[:], den[:])
                nc.vector.tensor_mul(aot[h][:, cs], av[:], rc[:])

            # ---------------- output projection for chunk c ----------------
            for st_i in range(4 * c, 4 * c + 4):
                ss = slice(st_i * 128, (st_i + 1) * 128)
                for hc in range(NSC):
                    hs = slice(hc * 512, (hc + 1) * 512)
                    ops = pb.tile([128, 512], f32, tag="pb", name="ops")
                    for m in range(REP):
                        nc.tensor.matmul(
                            ops[:],
                            lhsT=aot[m][:, ss],
                            rhs=wo_p[m][:, 0, hs],
                            start=(m == 0),
                            stop=(m == REP - 1),
                        )
                    oc = ocp.tile([128, 512], bf16, tag="oc", name="oc")
                    nc.vector.tensor_copy(oc[:], ops[:])
                    nc.sync.dma_start(out[ss, hs], oc[:])


def build_program():
    import concourse.tile as tile
    from concourse import bacc, mybir

    f32 = mybir.dt.float32
    bf16 = mybir.dt.bfloat16
    nc = bacc.Bacc("TRN2", target_bir_lowering=False, debug=False, num_devices=8)
    aps = {}
    aps["xT"] = nc.dram_tensor(
        "xT", [128, 4, 4, 4, 512], bf16, kind="ExternalInput"
    ).ap()
    aps["cosT"] = nc.dram_tensor("cosT", [D, S], bf16, kind="ExternalInput").ap()
    aps["sinT"] = nc.dram_tensor("sinT", [D, S], bf16, kind="ExternalInput").ap()
    aps["wq"] = nc.dram_tensor("wq", [128, 16, 512], bf16, kind="ExternalInput").ap()
    aps["wk"] = nc.dram_tensor("wk", [128, 16, 128], bf16, kind="ExternalInput").ap()
    aps["wv"] = nc.dram_tensor("wv", [128, 16, 128], bf16, kind="ExternalInput").ap()
    aps["wo"] = nc.dram_tensor("wo", [128, 4, 2048], bf16, kind="ExternalInput").ap()
    aps["mask"] = nc.dram_tensor("mask", [128, 128], f32, kind="ExternalInput").ap()
    aps["ones"] = nc.dram_tensor("ones", [128, 128], bf16, kind="ExternalInput").ap()
    aps["ones8"] = nc.dram_tensor(
        "ones8", [128, 256], mybir.dt.float8e4, kind="ExternalInput"
    ).ap()
    aps["out"] = nc.dram_tensor("out", [S, HID], bf16, kind="ExternalOutput").ap()

    with tile.TileContext(nc) as tc:
        _emit(nc, tc, aps)
    nc.compile()
    return nc


def _tile_kdim(w):
    """[K, M] -> [128, K//128, M] with element (p, t, m) = w[t*128+p, m]."""
    K, M = w.shape
    return np.ascontiguousarray(w.reshape(K // 128, 128, M).transpose(1, 0, 2))


def make_in_maps(x, cos, sin, Wq, Wk, Wv, Wo):
    """Build the 8 per-core input dicts. Core c: batch c//4, kv-group c%4."""
    import ml_dtypes

    bf = ml_dtypes.bfloat16
    mask = np.where(
        np.arange(128)[:, None] <= np.arange(128)[None, :], 0.0, NEG
    ).astype(np.float32)
    ones = np.ones((128, 128), dtype=bf)
    ones8 = np.ones((128, 256), dtype=ml_dtypes.float8_e4m3)
    # x^T pre-tiled: [128, chunk(4), piece(4), t_in_piece(4), s_in_chunk(512)]
    # with hid = (piece*4 + t)*128 + p and s = chunk*512 + s'.
    xT = []
    for b in range(B):
        A = np.ascontiguousarray(x[b].T).astype(bf)  # [2048 hid, 2048 s]
        A = A.reshape(4, 4, 128, 4, 512)  # [piece, t, p, chunk, s']
        xT.append(np.ascontiguousarray(A.transpose(2, 3, 0, 1, 4)))
    cosT = np.ascontiguousarray(cos.T).astype(bf)
    sinT = np.ascontiguousarray(sin.T).astype(np.float32)
    sinT[0:64, :] *= -1.0  # sign-fold rotate_half's negation into the table
    sinT = sinT.astype(bf)
    in_maps = []
    for c in range(8):
        b, g = c // 4, c % 4
        in_maps.append(
            {
                "xT": xT[b],
                "cosT": cosT,
                "sinT": sinT,
                "wq": _tile_kdim(Wq[:, g * REP * D : (g + 1) * REP * D]).astype(bf),
                "wk": _tile_kdim(Wk[:, g * D : (g + 1) * D]).astype(bf),
                "wv": _tile_kdim(Wv[:, g * D : (g + 1) * D]).astype(bf),
                "wo": _tile_kdim(Wo[g * REP * D : (g + 1) * REP * D, :]).astype(bf),
                "mask": mask,
                "ones": ones,
                "ones8": ones8,
            }
        )
    return in_maps


def kernel(x, cos, sin, Wq, Wk, Wv, Wo):
    from concourse import bass_utils

    nc = build_program()
    in_maps = make_in_maps(x, cos, sin, Wq, Wk, Wv, Wo)
    trace = bool(int(os.environ.get("BASS_KERNEL_TRACE", "0")))
    res = bass_utils.run_bass_kernel_spmd(
        nc,
        in_maps,
        core_ids=list(range(8)),
        trace=trace,
    )
    if trace:
        print(f"HW exec time: {res.exec_time_ns} ns")
        if res.instructions_and_trace is not None:
            print(f"trace: {res.instructions_and_trace[1]}")
    out = np.empty((B, S, HID), dtype=np.float32)
    for b in range(B):
        acc = res.results[4 * b]["out"].astype(np.float32)
        for g in range(1, G):
            acc = acc + res.results[4 * b + g]["out"].astype(np.float32)
        out[b] = acc
    return out


# revision 34
# speedup vs baseline: 1.0090x; 1.0090x over previous
"""Trainium2 Bass kernel for GQA attention (B=2, S=2048, HID=2048, H=16, G=4, D=128).

Sharding: 8 cores = 2 batches x 4 kv-groups. Core c handles batch c//4 and
kv-group c%4 (4 q heads + 1 kv head). Wq/Wk/Wv column-sharded by head group,
Wo row-sharded; per-core partial outputs are summed on the host per batch
(the unshard / all-reduce step).

v5: bf16 operands; fully pipelined per-chunk schedule (proj_c -> attn_c ->
outproj_c, k/v pass first); V projected directly in transposed layout
(lhsT = x); full-partition RoPE with sign-folded sin table; all DRAM operands
host-pre-tiled into SBUF layout so every DMA moves 2-4KB contiguous runs per
partition; bf16 output summed on the host in fp32.
"""

import os
import sys

sys.path.insert(0, "/opt/trn_rl_repo")

import numpy as np

B, S, HID = 2, 2048, 2048
H, G, D = 16, 4, 128
REP = H // G  # q heads per kv head = 4
NEG = -1e30
SCALE = 1.0 / np.sqrt(np.float32(D))

NKT = HID // 128  # 16 contraction tiles for projections
NSC = S // 512  # 4 s-chunks of 512
NST = S // 128  # 16 s-tiles of 128


def _emit(nc, tc, aps):
    """Emit the per-core program. aps: dict of DRAM APs."""
    from contextlib import ExitStack

    from concourse import mybir

    f32 = mybir.dt.float32
    bf16 = mybir.dt.bfloat16
    fp8 = mybir.dt.float8e4
    DR = mybir.MatmulPerfMode.DoubleRow
    Exp = mybir.ActivationFunctionType.Exp

    # pre-tiled DRAM operands (partition dim first, contiguous free dims)
    xT = aps["xT"]  # [128, 4c, 4p, 4t, 512] bf16
    wq, wk, wv, wo = aps["wq"], aps["wk"], aps["wv"], aps["wo"]
    cosT, sinT = aps["cosT"], aps["sinT"]  # [128, 2048] bf16
    mask, ones, ones8 = aps["mask"], aps["ones"], aps["ones8"]
    out = aps["out"]

    lo = slice(0, 64)
    hi = slice(64, 128)

    with ExitStack() as ctx:
        persist = ctx.enter_context(tc.tile_pool(name="persist", bufs=1))

        # constants
        mask_t = persist.tile([128, 128], f32, tag="mask", name="mask_t")
        ones_t = persist.tile([128, 128], bf16, tag="ones", name="ones_t")
        ones8_t = persist.tile([128, 2, 128], fp8, tag="ones8", name="ones8_t")

        # weights
        wq_p = [
            persist.tile([128, 4, 512], bf16, tag=f"wq{i}", name=f"wq{i}")
            for i in range(4)
        ]
        wk_sb = persist.tile([128, 16, 128], bf16, tag="wk", name="wk_sb")
        wv_sb = persist.tile([128, 16, 128], bf16, tag="wv", name="wv_sb")
        wo_p = [
            persist.tile([128, 1, 2048], bf16, tag=f"wo{i}", name=f"wo{i}")
            for i in range(4)
        ]
        cos_p = [
            persist.tile([128, 1024], bf16, tag=f"cos{i}", name=f"cos{i}")
            for i in range(2)
        ]
        sin_p = [
            persist.tile([128, 1024], bf16, tag=f"sin{i}", name=f"sin{i}")
            for i in range(2)
        ]

        # persistent activations
        qt = [
            persist.tile([128, S], bf16, tag=f"qt{h}", name=f"qt{h}")
            for h in range(REP)
        ]
        kt = persist.tile([128, S], bf16, tag="kt", name="kt")
        vs = persist.tile([128, NST, 128], bf16, tag="vs", name="vs")
        aot = [
            persist.tile([128, S], bf16, tag=f"aot{h}", name=f"aot{h}")
            for h in range(REP)
        ]

        xpool = ctx.enter_context(tc.tile_pool(name="xsl", bufs=8))
        quad = ctx.enter_context(tc.tile_pool(name="quad", bufs=4, space="PSUM"))
        pb = ctx.enter_context(tc.tile_pool(name="pb", bufs=4, space="PSUM"))
        epool = ctx.enter_context(tc.tile_pool(name="es", bufs=5))
        e8pool = ctx.enter_context(tc.tile_pool(name="es8", bufs=4))
        rpool = ctx.enter_context(tc.tile_pool(name="rope", bufs=4))
        rcp = ctx.enter_context(tc.tile_pool(name="rc", bufs=2))
        ocp = ctx.enter_context(tc.tile_pool(name="oc", bufs=4))

        # ---------------- startup DMA sequence ----------------
        # x slabs: per chunk, 4 piece-tiles of [128, 4, 512]
        xs = [
            [
                xpool.tile([128, 4, 512], bf16, tag="x", name=f"xs{c}_{p}")
                for p in range(4)
            ]
            for c in range(NSC)
        ]

        def dma_x_chunk(c, eng):
            for p in range(4):
                eng.dma_start(xs[c][p][:], xT[:, c, p, :, :])

        def dma_x_piece(c, p, eng):
            eng.dma_start(xs[c][p][:], xT[:, c, p, :, :])

        # scalar queue: wk/wv (gate the first pass), trig, mask
        nc.scalar.dma_start(wk_sb[:], wk)
        nc.scalar.dma_start(wv_sb[:], wv)
        nc.scalar.dma_start(cos_p[0][:], cosT[:, 0:1024])
        nc.scalar.dma_start(sin_p[0][:], sinT[:, 0:1024])
        nc.scalar.dma_start(cos_p[1][:], cosT[:, 1024:2048])
        nc.scalar.dma_start(sin_p[1][:], sinT[:, 1024:2048])
        nc.scalar.dma_start(mask_t[:], mask)
        # sync queue: x chunk0, first wq halves
        dma_x_chunk(0, nc.sync)
        nc.sync.dma_start(wq_p[0][:], wq[:, 0:4, :])
        nc.sync.dma_start(wq_p[1][:], wq[:, 4:8, :])
        # gpsimd queue: rest of wq, ones, x chunk1, wo
        nc.gpsimd.dma_start(wq_p[2][:], wq[:, 8:12, :])
        nc.gpsimd.dma_start(wq_p[3][:], wq[:, 12:16, :])
        nc.gpsimd.dma_start(ones_t[:], ones)
        nc.gpsimd.dma_start(ones8_t[:], ones8)
        dma_x_chunk(1, nc.gpsimd)
        for m in range(4):
            nc.gpsimd.dma_start(wo_p[m][:], wo[:, m : m + 1, :])

        def rope(acc, dest, cs_off, piece):
            """dest = acc*cos + swap_half(acc)*sin_signed  (sin pre-negated in
            its low half on the host)."""
            cp = cos_p[piece]
            sp_ = sin_p[piece]
            co = slice(cs_off, cs_off + 512)
            tmp_a = rpool.tile([128, 512], f32, tag="ra", name="tmp_a")
            tmp_b = rpool.tile([128, 512], f32, tag="rb", name="tmp_b")
            nc.vector.tensor_mul(tmp_b[lo, :], acc[hi, :], sp_[lo, co])
            nc.vector.tensor_mul(tmp_b[hi, :], acc[lo, :], sp_[hi, co])
            nc.vector.tensor_mul(tmp_a[:], acc[:], cp[:, co])
            nc.vector.tensor_add(dest, tmp_a[:], tmp_b[:])

        for c in range(NSC):
            cs = slice(c * 512, (c + 1) * 512)
            piece = c // 2
            cs_off = (c % 2) * 512

            # ---------------- projections ----------------
            # pass A: k and v^T (so RoPE-k and V land first for attn)
            k_acc = quad.tile([128, 512], f32, tag="qd", name="k_acc")
            for t in range(NKT):
                nc.tensor.matmul(
                    k_acc[:],
                    lhsT=wk_sb[:, t, :],
                    rhs=xs[c][t // 4][:, t % 4, :],
                    start=(t == 0),
                    stop=(t == NKT - 1),
                )
            vT_acc = quad.tile([128, 4, 128], f32, tag="qd", name="vT_acc")
            for i in range(4):
                for t in range(NKT):
                    nc.tensor.matmul(
                        vT_acc[:, i, :],
                        lhsT=xs[c][t // 4][:, t % 4, i * 128 : (i + 1) * 128],
                        rhs=wv_sb[:, t, :],
                        start=(t == 0),
                        stop=(t == NKT - 1),
                    )
            rope(k_acc, kt[:, cs], cs_off, piece)
            nc.scalar.copy(vs[:, 4 * c : 4 * c + 4, :], vT_acc[:])

            # pass B: q0, q1 ; pass C: q2, q3
            for pair in range(2):
                q_acc = [
                    quad.tile([128, 512], f32, tag="qd", name=f"q_acc{pair}{j}")
                    for j in range(2)
                ]
                for t in range(NKT):
                    for j in range(2):
                        h = 2 * pair + j
                        nc.tensor.matmul(
                            q_acc[j][:],
                            lhsT=wq_p[t // 4][:, t % 4, h * 128 : (h + 1) * 128],
                            rhs=xs[c][t // 4][:, t % 4, :],
                            start=(t == 0),
                            stop=(t == NKT - 1),
                        )
                for j in range(2):
                    h = 2 * pair + j
                    rope(q_acc[j], qt[h][:, cs], cs_off, piece)

            # prefetch x for chunk c+2
            if c + 2 < NSC:
                dma_x_chunk(c + 2, nc.sync)

            # ---------------- attention for q-chunk c ----------------
            for h in range(REP):
                av = pb.tile([128, 512], f32, tag="pb", name="av")
                den = pb.tile([128, 512], f32, tag="pb", name="den")
                es_slabs = []
                es8_slabs = []
                for kb in range(c + 1):
                    diag = kb == c
                    es = epool.tile([128, 4, 512], bf16, tag="es", name="es")
                    es_slabs.append(es)
                    for j in range(4):
                        i = kb * 4 + j
                        j0 = j * 128 if diag else 0
                        sp_t = pb.tile([128, 512], f32, tag="pb", name="sp_t")
                        nc.tensor.matmul(
                            sp_t[:, j0:512],
                            lhsT=kt[:, i * 128 : (i + 1) * 128],
                            rhs=qt[h][:, c * 512 + j0 : (c + 1) * 512],
                            start=True,
                            stop=True,
                        )
                        if diag:
                            nc.vector.tensor_add(
                                sp_t[:, j * 128 : (j + 1) * 128],
                                sp_t[:, j * 128 : (j + 1) * 128],
                                mask_t[:],
                            )
                        nc.scalar.activation(
                            es[:, j, j0:512],
                            sp_t[:, j0:512],
                            Exp,
                            scale=float(SCALE),
                        )
                    if not diag:
                        # fp8 shadow copy for the DoubleRow denominator; cast
                        # engine alternates so the trace ranks their speed
                        es8 = e8pool.tile([128, 4, 512], fp8, tag="es8", name="es8")
                        es8_slabs.append(es8)
                        nc.vector.tensor_copy(es8[:], es[:])
                    for j in range(4):
                        i = kb * 4 + j
                        j0 = j * 128 if diag else 0
                        nc.tensor.matmul(
                            av[:, j0:512],
                            lhsT=vs[:, i, :],
                            rhs=es[:, j, j0:512],
                            start=(i == 0),
                            stop=(i == 4 * c + 3),
                        )
                # denominator phase: DoubleRow fp8 for full groups, bf16 for
                # the diagonal; kept off the sc/av critical path
                for kb in range(c):
                    for pair in range(2):
                        nc.tensor.matmul(
                            den[:, :],
                            lhsT=ones8_t[:],
                            rhs=es8_slabs[kb][:, 2 * pair : 2 * pair + 2, :],
                            start=(kb == 0 and pair == 0),
                            stop=False,
                            perf_mode=DR,
                        )
                for j in range(4):
                    j0 = j * 128
                    nc.tensor.matmul(
                        den[:, j0:512],
                        lhsT=ones_t[:],
                        rhs=es_slabs[c][:, j, j0:512],
                        start=(c == 0 and j == 0),
                        stop=(j == 3),
                    )
                rc = rcp.tile([128, 512], f32, tag="rc", name="rc")
                nc.vector.reciprocal_approx_fast(rc[:], den[:])
                nc.vector.tensor_mul(aot[h][:, cs], av[:], rc[:])

            # ---------------- output projection for chunk c ----------------
            for st_i in range(4 * c, 4 * c + 4):
                ss = slice(st_i * 128, (st_i + 1) * 128)
                for hc in range(NSC):
                    hs = slice(hc * 512, (hc + 1) * 512)
                    ops = pb.tile([128, 512], f32, tag="pb", name="ops")
                    for m in range(REP):
                        nc.tensor.matmul(
                            ops[:],
                            lhsT=aot[m][:, ss],
                            rhs=wo_p[m][:, 0, hs],
                            start=(m == 0),
                            stop=(m == REP - 1),
                        )
                    oc = ocp.tile([128, 512], bf16, tag="oc", name="oc")
                    nc.vector.tensor_copy(oc[:], ops[:])
                    nc.sync.dma_start(out[ss, hs], oc[:])


def build_program():
    import concourse.tile as tile
    from concourse import bacc, mybir

    f32 = mybir.dt.float32
    bf16 = mybir.dt.bfloat16
    nc = bacc.Bacc("TRN2", target_bir_lowering=False, debug=False, num_devices=8)
    aps = {}
    aps["xT"] = nc.dram_tensor(
        "xT", [128, 4, 4, 4, 512], bf16, kind="ExternalInput"
    ).ap()
    aps["cosT"] = nc.dram_tensor("cosT", [D, S], bf16, kind="ExternalInput").ap()
    aps["sinT"] = nc.dram_tensor("sinT", [D, S], bf16, kind="ExternalInput").ap()
    aps["wq"] = nc.dram_tensor("wq", [128, 16, 512], bf16, kind="ExternalInput").ap()
    aps["wk"] = nc.dram_tensor("wk", [128, 16, 128], bf16, kind="ExternalInput").ap()
    aps["wv"] = nc.dram_tensor("wv", [128, 16, 128], bf16, kind="ExternalInput").ap()
    aps["wo"] = nc.dram_tensor("wo", [128, 4, 2048], bf16, kind="ExternalInput").ap()
    aps["mask"] = nc.dram_tensor("mask", [128, 128], f32, kind="ExternalInput").ap()
    aps["ones"] = nc.dram_tensor("ones", [128, 128], bf16, kind="ExternalInput").ap()
    aps["ones8"] = nc.dram_tensor(
        "ones8", [128, 256], mybir.dt.float8e4, kind="ExternalInput"
    ).ap()
    aps["out"] = nc.dram_tensor("out", [S, HID], bf16, kind="ExternalOutput").ap()

    with tile.TileContext(nc) as tc:
        _emit(nc, tc, aps)
    nc.compile()
    return nc


def _tile_kdim(w):
    """[K, M] -> [128, K//128, M] with element (p, t, m) = w[t*128+p, m]."""
    K, M = w.shape
    return np.ascontiguousarray(w.reshape(K // 128, 128, M).transpose(1, 0, 2))


def make_in_maps(x, cos, sin, Wq, Wk, Wv, Wo):
    """Build the 8 per-core input dicts. Core c: batch c//4, kv-group c%4."""
    import ml_dtypes

    bf = ml_dtypes.bfloat16
    mask = np.where(
        np.arange(128)[:, None] <= np.arange(128)[None, :], 0.0, NEG
    ).astype(np.float32)
    ones = np.ones((128, 128), dtype=bf)
    ones8 = np.ones((128, 256), dtype=ml_dtypes.float8_e4m3)
    # x^T pre-tiled: [128, chunk(4), piece(4), t_in_piece(4), s_in_chunk(512)]
    # with hid = (piece*4 + t)*128 + p and s = chunk*512 + s'.
    xT = []
    for b in range(B):
        A = np.ascontiguousarray(x[b].T).astype(bf)  # [2048 hid, 2048 s]
        A = A.reshape(4, 4, 128, 4, 512)  # [piece, t, p, chunk, s']
        xT.append(np.ascontiguousarray(A.transpose(2, 3, 0, 1, 4)))
    cosT = np.ascontiguousarray(cos.T).astype(bf)
    sinT = np.ascontiguousarray(sin.T).astype(np.float32)
    sinT[0:64, :] *= -1.0  # sign-fold rotate_half's negation into the table
    sinT = sinT.astype(bf)
    in_maps = []
    for c in range(8):
        b, g = c // 4, c % 4
        in_maps.append(
            {
                "xT": xT[b],
                "cosT": cosT,
                "sinT": sinT,
                "wq": _tile_kdim(Wq[:, g * REP * D : (g + 1) * REP * D]).astype(bf),
                "wk": _tile_kdim(Wk[:, g * D : (g + 1) * D]).astype(bf),
                "wv": _tile_kdim(Wv[:, g * D : (g + 1) * D]).astype(bf),
                "wo": _tile_kdim(Wo[g * REP * D : (g + 1) * REP * D, :]).astype(bf),
                "mask": mask,
                "ones": ones,
                "ones8": ones8,
            }
        )
    return in_maps


def kernel(x, cos, sin, Wq, Wk, Wv, Wo):
    from concourse import bass_utils

    nc = build_program()
    in_maps = make_in_maps(x, cos, sin, Wq, Wk, Wv, Wo)
    trace = bool(int(os.environ.get("BASS_KERNEL_TRACE", "0")))
    res = bass_utils.run_bass_kernel_spmd(
        nc,
        in_maps,
        core_ids=list(range(8)),
        trace=trace,
    )
    if trace:
        print(f"HW exec time: {res.exec_time_ns} ns")
        if res.instructions_and_trace is not None:
            print(f"trace: {res.instructions_and_trace[1]}")
    out = np.empty((B, S, HID), dtype=np.float32)
    for b in range(B):
        acc = res.results[4 * b]["out"].astype(np.float32)
        for g in range(1, G):
            acc = acc + res.results[4 * b + g]["out"].astype(np.float32)
        out[b] = acc
    return out


# revision 37
# speedup vs baseline: 1.0241x; 1.0150x over previous
"""Trainium2 Bass kernel for GQA attention (B=2, S=2048, HID=2048, H=16, G=4, D=128).

Sharding: 8 cores = 2 batches x 4 kv-groups. Core c handles batch c//4 and
kv-group c%4 (4 q heads + 1 kv head). Wq/Wk/Wv column-sharded by head group,
Wo row-sharded; per-core partial outputs are summed on the host per batch
(the unshard / all-reduce step).

v5: bf16 operands; fully pipelined per-chunk schedule (proj_c -> attn_c ->
outproj_c, k/v pass first); V projected directly in transposed layout
(lhsT = x); full-partition RoPE with sign-folded sin table; all DRAM operands
host-pre-tiled into SBUF layout so every DMA moves 2-4KB contiguous runs per
partition; bf16 output summed on the host in fp32.
"""

import os
import sys

sys.path.insert(0, "/opt/trn_rl_repo")

import numpy as np

B, S, HID = 2, 2048, 2048
H, G, D = 16, 4, 128
REP = H // G  # q heads per kv head = 4
NEG = -1e30
SCALE = 1.0 / np.sqrt(np.float32(D))

NKT = HID // 128  # 16 contraction tiles for projections
NSC = S // 512  # 4 s-chunks of 512
NST = S // 128  # 16 s-tiles of 128


def _emit(nc, tc, aps):
    """Emit the per-core program. aps: dict of DRAM APs."""
    from contextlib import ExitStack

    from concourse import mybir

    f32 = mybir.dt.float32
    bf16 = mybir.dt.bfloat16
    fp8 = mybir.dt.float8e4
    DR = mybir.MatmulPerfMode.DoubleRow
    Exp = mybir.ActivationFunctionType.Exp

    # pre-tiled DRAM operands (partition dim first, contiguous free dims)
    xT = aps["xT"]  # [128, 4c, 4p, 4t, 512] bf16
    wq, wk, wv, wo = aps["wq"], aps["wk"], aps["wv"], aps["wo"]
    cosT, sinT = aps["cosT"], aps["sinT"]  # [128, 2048] bf16
    mask, ones, ones8 = aps["mask"], aps["ones"], aps["ones8"]
    out = aps["out"]

    lo = slice(0, 64)
    hi = slice(64, 128)

    with ExitStack() as ctx:
        persist = ctx.enter_context(tc.tile_pool(name="persist", bufs=1))

        # constants
        mask_t = persist.tile([128, 128], f32, tag="mask", name="mask_t")
        ones_t = persist.tile([128, 128], bf16, tag="ones", name="ones_t")
        ones8_t = persist.tile([128, 2, 128], fp8, tag="ones8", name="ones8_t")

        # weights
        wq_p = [
            persist.tile([128, 4, 512], bf16, tag=f"wq{i}", name=f"wq{i}")
            for i in range(4)
        ]
        wk_sb = persist.tile([128, 16, 128], bf16, tag="wk", name="wk_sb")
        wv_sb = persist.tile([128, 16, 128], bf16, tag="wv", name="wv_sb")
        wo_p = [
            persist.tile([128, 1, 2048], bf16, tag=f"wo{i}", name=f"wo{i}")
            for i in range(4)
        ]
        cos_p = [
            persist.tile([128, 1024], bf16, tag=f"cos{i}", name=f"cos{i}")
            for i in range(2)
        ]
        sin_p = [
            persist.tile([128, 1024], bf16, tag=f"sin{i}", name=f"sin{i}")
            for i in range(2)
        ]

        # persistent activations
        qt = [
            persist.tile([128, S], bf16, tag=f"qt{h}", name=f"qt{h}")
            for h in range(REP)
        ]
        kt = persist.tile([128, S], bf16, tag="kt", name="kt")
        vs = persist.tile([128, NST, 128], bf16, tag="vs", name="vs")
        aot = [
            persist.tile([128, S], bf16, tag=f"aot{h}", name=f"aot{h}")
            for h in range(REP)
        ]

        xpool = ctx.enter_context(tc.tile_pool(name="xsl", bufs=8))
        quad = ctx.enter_context(tc.tile_pool(name="quad", bufs=4, space="PSUM"))
        pb = ctx.enter_context(tc.tile_pool(name="pb", bufs=4, space="PSUM"))
        epool = ctx.enter_context(tc.tile_pool(name="es", bufs=5))
        e8pool = ctx.enter_context(tc.tile_pool(name="es8", bufs=4))
        rpool = ctx.enter_context(tc.tile_pool(name="rope", bufs=4))
        rcp = ctx.enter_context(tc.tile_pool(name="rc", bufs=2))
        ocp = ctx.enter_context(tc.tile_pool(name="oc", bufs=4))

        # ---------------- startup DMA sequence ----------------
        # x slabs: per chunk, 4 piece-tiles of [128, 4, 512]
        xs = [
            [
                xpool.tile([128, 4, 512], bf16, tag="x", name=f"xs{c}_{p}")
                for p in range(4)
            ]
            for c in range(NSC)
        ]

        def dma_x_chunk(c, eng):
            for p in range(4):
                eng.dma_start(xs[c][p][:], xT[:, c, p, :, :])

        def dma_x_piece(c, p, eng):
            eng.dma_start(xs[c][p][:], xT[:, c, p, :, :])

        # scalar queue: wk + first wq piece (gate chunk-0's merged pass),
        # wv, trig, mask
        nc.scalar.dma_start(wk_sb[:], wk)
        nc.scalar.dma_start(wq_p[0][:], wq[:, 0:4, :])
        nc.scalar.dma_start(wv_sb[:], wv)
        nc.scalar.dma_start(cos_p[0][:], cosT[:, 0:1024])
        nc.scalar.dma_start(sin_p[0][:], sinT[:, 0:1024])
        nc.scalar.dma_start(cos_p[1][:], cosT[:, 1024:2048])
        nc.scalar.dma_start(sin_p[1][:], sinT[:, 1024:2048])
        nc.scalar.dma_start(mask_t[:], mask)
        # sync queue: x chunk0
        dma_x_chunk(0, nc.sync)
        # gpsimd queue: rest of wq (arrive piece-by-piece for the merged
        # pass), ones, x chunk1, wo
        nc.gpsimd.dma_start(wq_p[1][:], wq[:, 4:8, :])
        nc.gpsimd.dma_start(wq_p[2][:], wq[:, 8:12, :])
        nc.gpsimd.dma_start(wq_p[3][:], wq[:, 12:16, :])
        nc.gpsimd.dma_start(ones_t[:], ones)
        nc.gpsimd.dma_start(ones8_t[:], ones8)
        dma_x_chunk(1, nc.gpsimd)
        for m in range(4):
            nc.gpsimd.dma_start(wo_p[m][:], wo[:, m : m + 1, :])

        def rope(acc, dest, cs_off, piece):
            """dest = acc*cos + swap_half(acc)*sin_signed  (sin pre-negated in
            its low half on the host)."""
            cp = cos_p[piece]
            sp_ = sin_p[piece]
            co = slice(cs_off, cs_off + 512)
            tmp_a = rpool.tile([128, 512], f32, tag="ra", name="tmp_a")
            tmp_b = rpool.tile([128, 512], f32, tag="rb", name="tmp_b")
            nc.vector.tensor_mul(tmp_b[lo, :], acc[hi, :], sp_[lo, co])
            nc.vector.tensor_mul(tmp_b[hi, :], acc[lo, :], sp_[hi, co])
            nc.vector.tensor_mul(tmp_a[:], acc[:], cp[:, co])
            nc.vector.tensor_add(dest, tmp_a[:], tmp_b[:])

        for c in range(NSC):
            cs = slice(c * 512, (c + 1) * 512)
            piece = c // 2
            cs_off = (c % 2) * 512

            # ---------------- projections ----------------
            if c == 0:
                # chunk 0 is paced by the x-piece DMAs: interleave k with
                # q0/q1 per piece (3 open accumulation groups, one bank each)
                # so the PE has ~2.6us of work per 2.5us piece arrival
                k_acc = quad.tile([128, 512], f32, tag="qd", name="k_acc")
                q01 = [
                    quad.tile([128, 512], f32, tag="qd", name=f"q_acc0{j}")
                    for j in range(2)
                ]
                for p in range(4):
                    for tt in range(4):
                        t = 4 * p + tt
                        nc.tensor.matmul(
                            k_acc[:],
                            lhsT=wk_sb[:, t, :],
                            rhs=xs[0][p][:, tt, :],
                            start=(t == 0),
                            stop=(t == NKT - 1),
                        )
                    for tt in range(4):
                        t = 4 * p + tt
                        for j in range(2):
                            nc.tensor.matmul(
                                q01[j][:],
                                lhsT=wq_p[p][:, tt, j * 128 : (j + 1) * 128],
                                rhs=xs[0][p][:, tt, :],
                                start=(t == 0),
                                stop=(t == NKT - 1),
                            )
                rope(k_acc, kt[:, cs], cs_off, piece)
                vT_acc = quad.tile([128, 4, 128], f32, tag="qd", name="vT_acc")
                for i in range(4):
                    for t in range(NKT):
                        nc.tensor.matmul(
                            vT_acc[:, i, :],
                            lhsT=xs[0][t // 4][:, t % 4, i * 128 : (i + 1) * 128],
                            rhs=wv_sb[:, t, :],
                            start=(t == 0),
                            stop=(t == NKT - 1),
                        )
                rope(q01[0], qt[0][:, cs], cs_off, piece)
                rope(q01[1], qt[1][:, cs], cs_off, piece)
                nc.scalar.copy(vs[:, 0:4, :], vT_acc[:])
                pairs = [1]
            else:
                # pass A: k and v^T (so RoPE-k and V land first for attn)
                k_acc = quad.tile([128, 512], f32, tag="qd", name="k_acc")
                for t in range(NKT):
                    nc.tensor.matmul(
                        k_acc[:],
                        lhsT=wk_sb[:, t, :],
                        rhs=xs[c][t // 4][:, t % 4, :],
                        start=(t == 0),
                        stop=(t == NKT - 1),
                    )
                vT_acc = quad.tile([128, 4, 128], f32, tag="qd", name="vT_acc")
                for i in range(4):
                    for t in range(NKT):
                        nc.tensor.matmul(
                            vT_acc[:, i, :],
                            lhsT=xs[c][t // 4][:, t % 4, i * 128 : (i + 1) * 128],
                            rhs=wv_sb[:, t, :],
                            start=(t == 0),
                            stop=(t == NKT - 1),
                        )
                rope(k_acc, kt[:, cs], cs_off, piece)
                nc.scalar.copy(vs[:, 4 * c : 4 * c + 4, :], vT_acc[:])
                pairs = [0, 1]

            # remaining q pairs
            for pair in pairs:
                q_acc = [
                    quad.tile([128, 512], f32, tag="qd", name=f"q_acc{pair}{j}")
                    for j in range(2)
                ]
                for t in range(NKT):
                    for j in range(2):
                        h = 2 * pair + j
                        nc.tensor.matmul(
                            q_acc[j][:],
                            lhsT=wq_p[t // 4][:, t % 4, h * 128 : (h + 1) * 128],
                            rhs=xs[c][t // 4][:, t % 4, :],
                            start=(t == 0),
                            stop=(t == NKT - 1),
                        )
                for j in range(2):
                    h = 2 * pair + j
                    rope(q_acc[j], qt[h][:, cs], cs_off, piece)

            # prefetch x for chunk c+2
            if c + 2 < NSC:
                dma_x_chunk(c + 2, nc.sync)

            # ---------------- attention for q-chunk c ----------------
            for h in range(REP):
                av = pb.tile([128, 512], f32, tag="pb", name="av")
                den = pb.tile([128, 512], f32, tag="pb", name="den")
                es_slabs = []
                es8_slabs = []
                for kb in range(c + 1):
                    diag = kb == c
                    es = epool.tile([128, 4, 512], bf16, tag="es", name="es")
                    es_slabs.append(es)
                    for j in range(4):
                        i = kb * 4 + j
                        j0 = j * 128 if diag else 0
                        sp_t = pb.tile([128, 512], f32, tag="pb", name="sp_t")
                        nc.tensor.matmul(
                            sp_t[:, j0:512],
                            lhsT=kt[:, i * 128 : (i + 1) * 128],
                            rhs=qt[h][:, c * 512 + j0 : (c + 1) * 512],
                            start=True,
                            stop=True,
                        )
                        if diag:
                            nc.vector.tensor_add(
                                sp_t[:, j * 128 : (j + 1) * 128],
                                sp_t[:, j * 128 : (j + 1) * 128],
                                mask_t[:],
                            )
                        nc.scalar.activation(
                            es[:, j, j0:512],
                            sp_t[:, j0:512],
                            Exp,
                            scale=float(SCALE),
                        )
                    if not diag:
                        # fp8 shadow copy for the DoubleRow denominator; cast
                        # engine alternates so the trace ranks their speed
                        es8 = e8pool.tile([128, 4, 512], fp8, tag="es8", name="es8")
                        es8_slabs.append(es8)
                        nc.vector.tensor_copy(es8[:], es[:])
                    for j in range(4):
                        i = kb * 4 + j
                        j0 = j * 128 if diag else 0
                        nc.tensor.matmul(
                            av[:, j0:512],
                            lhsT=vs[:, i, :],
                            rhs=es[:, j, j0:512],
                            start=(i == 0),
                            stop=(i == 4 * c + 3),
                        )
                # denominator phase: DoubleRow fp8 for full groups, bf16 for
                # the diagonal; kept off the sc/av critical path
                for kb in range(c):
                    for pair in range(2):
                        nc.tensor.matmul(
                            den[:, :],
                            lhsT=ones8_t[:],
                            rhs=es8_slabs[kb][:, 2 * pair : 2 * pair + 2, :],
                            start=(kb == 0 and pair == 0),
                            stop=False,
                            perf_mode=DR,
                        )
                for j in range(4):
                    j0 = j * 128
                    nc.tensor.matmul(
                        den[:, j0:512],
                        lhsT=ones_t[:],
                        rhs=es_slabs[c][:, j, j0:512],
                        start=(c == 0 and j == 0),
                        stop=(j == 3),
                    )
                rc = rcp.tile([128, 512], f32, tag="rc", name="rc")
                nc.vector.reciprocal_approx_fast(rc[:], den[:])
                nc.vector.tensor_mul(aot[h][:, cs], av[:], rc[:])

            # ---------------- output projection for chunk c ----------------
            for st_i in range(4 * c, 4 * c + 4):
                ss = slice(st_i * 128, (st_i + 1) * 128)
                for hc in range(NSC):
                    hs = slice(hc * 512, (hc + 1) * 512)
                    ops = pb.tile([128, 512], f32, tag="pb", name="ops")
                    for m in range(REP):
                        nc.tensor.matmul(
                            ops[:],
                            lhsT=aot[m][:, ss],
                            rhs=wo_p[m][:, 0, hs],
                            start=(m == 0),
                            stop=(m == REP - 1),
                        )
                    oc = ocp.tile([128, 512], bf16, tag="oc", name="oc")
                    if c == NSC - 1 and (st_i + hc) % 2 == 0:
                        # last chunk: split copies with the now-idle scalar
                        # engine so the final drain isn't serialized on vector
                        nc.scalar.copy(oc[:], ops[:])
                    else:
                        nc.vector.tensor_copy(oc[:], ops[:])
                    nc.sync.dma_start(out[ss, hs], oc[:])


def build_program():
    import concourse.tile as tile
    from concourse import bacc, mybir

    f32 = mybir.dt.float32
    bf16 = mybir.dt.bfloat16
    nc = bacc.Bacc("TRN2", target_bir_lowering=False, debug=False, num_devices=8)
    aps = {}
    aps["xT"] = nc.dram_tensor(
        "xT", [128, 4, 4, 4, 512], bf16, kind="ExternalInput"
    ).ap()
    aps["cosT"] = nc.dram_tensor("cosT", [D, S], bf16, kind="ExternalInput").ap()
    aps["sinT"] = nc.dram_tensor("sinT", [D, S], bf16, kind="ExternalInput").ap()
    aps["wq"] = nc.dram_tensor("wq", [128, 16, 512], bf16, kind="ExternalInput").ap()
    aps["wk"] = nc.dram_tensor("wk", [128, 16, 128], bf16, kind="ExternalInput").ap()
    aps["wv"] = nc.dram_tensor("wv", [128, 16, 128], bf16, kind="ExternalInput").ap()
    aps["wo"] = nc.dram_tensor("wo", [128, 4, 2048], bf16, kind="ExternalInput").ap()
    aps["mask"] = nc.dram_tensor("mask", [128, 128], f32, kind="ExternalInput").ap()
    aps["ones"] = nc.dram_tensor("ones", [128, 128], bf16, kind="ExternalInput").ap()
    aps["ones8"] = nc.dram_tensor(
        "ones8", [128, 256], mybir.dt.float8e4, kind="ExternalInput"
    ).ap()
    aps["out"] = nc.dram_tensor("out", [S, HID], bf16, kind="ExternalOutput").ap()

    with tile.TileContext(nc) as tc:
        _emit(nc, tc, aps)
    nc.compile()
    return nc


def _tile_kdim(w):
    """[K, M] -> [128, K//128, M] with element (p, t, m) = w[t*128+p, m]."""
    K, M = w.shape
    return np.ascontiguousarray(w.reshape(K // 128, 128, M).transpose(1, 0, 2))


def make_in_maps(x, cos, sin, Wq, Wk, Wv, Wo):
    """Build the 8 per-core input dicts. Core c: batch c//4, kv-group c%4."""
    import ml_dtypes

    bf = ml_dtypes.bfloat16
    mask = np.where(
        np.arange(128)[:, None] <= np.arange(128)[None, :], 0.0, NEG
    ).astype(np.float32)
    ones = np.ones((128, 128), dtype=bf)
    ones8 = np.ones((128, 256), dtype=ml_dtypes.float8_e4m3)
    # x^T pre-tiled: [128, chunk(4), piece(4), t_in_piece(4), s_in_chunk(512)]
    # with hid = (piece*4 + t)*128 + p and s = chunk*512 + s'.
    xT = []
    for b in range(B):
        A = np.ascontiguousarray(x[b].T).astype(bf)  # [2048 hid, 2048 s]
        A = A.reshape(4, 4, 128, 4, 512)  # [piece, t, p, chunk, s']
        xT.append(np.ascontiguousarray(A.transpose(2, 3, 0, 1, 4)))
    cosT = np.ascontiguousarray(cos.T).astype(bf)
    sinT = np.ascontiguousarray(sin.T).astype(np.float32)
    sinT[0:64, :] *= -1.0  # sign-fold rotate_half's negation into the table
    sinT = sinT.astype(bf)
    in_maps = []
    for c in range(8):
        b, g = c // 4, c % 4
        in_maps.append(
            {
                "xT": xT[b],
                "cosT": cosT,
                "sinT": sinT,
                "wq": _tile_kdim(Wq[:, g * REP * D : (g + 1) * REP * D]).astype(bf),
                "wk": _tile_kdim(Wk[:, g * D : (g + 1) * D]).astype(bf),
                "wv": _tile_kdim(Wv[:, g * D : (g + 1) * D]).astype(bf),
                "wo": _tile_kdim(Wo[g * REP * D : (g + 1) * REP * D, :]).astype(bf),
                "mask": mask,
                "ones": ones,
                "ones8": ones8,
            }
        )
    return in_maps


def kernel(x, cos, sin, Wq, Wk, Wv, Wo):
    from concourse import bass_utils

    nc = build_program()
    in_maps = make_in_maps(x, cos, sin, Wq, Wk, Wv, Wo)
    trace = bool(int(os.environ.get("BASS_KERNEL_TRACE", "0")))
    res = bass_utils.run_bass_kernel_spmd(
        nc,
        in_maps,
        core_ids=list(range(8)),
        trace=trace,
    )
    if trace:
        print(f"HW exec time: {res.exec_time_ns} ns")
        if res.instructions_and_trace is not None:
            print(f"trace: {res.instructions_and_trace[1]}")
    out = np.empty((B, S, HID), dtype=np.float32)
    for b in range(B):
        acc = res.results[4 * b]["out"].astype(np.float32)
        for g in range(1, G):
            acc = acc + res.results[4 * b + g]["out"].astype(np.float32)
        out[b] = acc
    return out
